# revision 16
# baseline (speedup 1.0000x reference)
"""Trainium2 Bass kernel for the nn_Attention problem.

Math (per flattened batch row b of x):
    qkv = x @ W_pre + b_pre                  # [B, 3*16*128]
    q,k,v -> [B, 16, 128]
    S = softmax(q k^T / sqrt(128), axis=g)   # [B, 16, 16]
    out = (sum_h S_h) . v @ W_proj + 16*b_proj
        = (sigma^T V) @ W_proj + 16*b_proj   with sigma[g] = sum_h S[h, g]

Implementation notes:
  - Data-parallel over 8 NeuronCores: 4096 rows/core (32 tiles of 128 rows).
  - bf16 matmuls with fp32 PSUM accumulation; softmax in fp32.
  - Attention processed in groups of 8 rows so the 128x128 PE array is full:
    stationary/moving operands are [d=128, (8 rows x 16 heads)] slices of
    DMA-transposed Q/K. The block-diagonal validity mask is applied as an
    additive -1e6 before exp.
  - sigma = E^T r computed on the PE (contract over the (row,head) partition
    dim), scattered to a block-diagonal [128, 8] operand, and contracted with
    V8 (v rows expanded onto partitions via an SBUF->SBUF reshape DMA) to
    accumulate C^T directly; C^T is then the stationary operand of the final
    projection matmul.
"""

import os
import sys

import numpy as np

for _p in ("/opt/trn_rl_repo",):
    if _p not in sys.path:
        sys.path.insert(0, _p)

import ml_dtypes  # noqa: E402

BF16 = ml_dtypes.bfloat16

HEAD_NUM = 16
HEAD_DIM = 128
INPUT_DIM = 1024
OUTPUT_DIM = 1024
QKV_DIM = 3 * HEAD_NUM * HEAD_DIM  # 6144
N_CORES = 8
B_TOTAL = 64 * 512
ROWS_PER_CORE = B_TOTAL // N_CORES  # 4096
P = 128

_PROG = None


def _build_program(n_tiles=ROWS_PER_CORE // P):
    from contextlib import ExitStack

    import concourse.tile as tile
    from concourse import bacc, mybir

    dt = mybir.dt
    Alu = mybir.AluOpType
    Act = mybir.ActivationFunctionType

    rows = n_tiles * P
    nc = bacc.Bacc("TRN2", target_bir_lowering=False, debug=False,
                   num_devices=N_CORES)

    x_d = nc.dram_tensor("x", [rows, INPUT_DIM], dt.bfloat16,
                         kind="ExternalInput")
    wpre_d = nc.dram_tensor("w_pre", [INPUT_DIM, QKV_DIM], dt.bfloat16,
                            kind="ExternalInput")
    bpre_d = nc.dram_tensor("b_pre_rep", [P, QKV_DIM], dt.bfloat16,
                            kind="ExternalInput")
    wproj_d = nc.dram_tensor("w_proj", [HEAD_DIM, OUTPUT_DIM], dt.bfloat16,
                             kind="ExternalInput")
    bproj_d = nc.dram_tensor("b_proj16_rep", [P, OUTPUT_DIM], dt.float32,
                             kind="ExternalInput")
    mask01_d = nc.dram_tensor("mask01", [P, P], dt.bfloat16,
                              kind="ExternalInput")
    mask8_d = nc.dram_tensor("mask8", [P, 8], dt.bfloat16,
                             kind="ExternalInput")
    ident_d = nc.dram_tensor("ident", [P, P], dt.bfloat16,
                             kind="ExternalInput")
    out_d = nc.dram_tensor("out", [rows, OUTPUT_DIM], dt.float32,
                           kind="ExternalOutput")

    KC = INPUT_DIM // P          # 8 contraction chunks
    NCHUNK = 512                 # psum free width
    SWEEP = 4                    # psum banks used by the qkv matmul
    NSWEEPS = QKV_DIM // (SWEEP * NCHUNK)  # 3
    GROUPS = P // 8              # 16 groups of 8 rows per tile
    INV_SQRT_D = 1.0 / float(np.sqrt(HEAD_DIM))

    with tile.TileContext(nc) as tc, ExitStack() as ctx:
        consts = ctx.enter_context(tc.tile_pool(name="consts", bufs=1))
        xt_pool = ctx.enter_context(tc.tile_pool(name="xt", bufs=2))
        qkv_pool = ctx.enter_context(tc.tile_pool(name="qkv", bufs=2))
        qt_pool = ctx.enter_context(tc.tile_pool(name="qt", bufs=2))
        kt_pool = ctx.enter_context(tc.tile_pool(name="kt", bufs=2))
        v8_pool = ctx.enter_context(tc.tile_pool(name="v8", bufs=2))
        att_pool = ctx.enter_context(tc.tile_pool(name="att", bufs=4))
        ct_pool = ctx.enter_context(tc.tile_pool(name="ct", bufs=2))
        out_pool = ctx.enter_context(tc.tile_pool(name="outp", bufs=2))

        qkv_ps = ctx.enter_context(
            tc.tile_pool(name="qkv_ps", bufs=SWEEP, space="PSUM"))
        z_ps = ctx.enter_context(tc.tile_pool(name="z_ps", bufs=3, space="PSUM"))
        ct_ps = ctx.enter_context(tc.tile_pool(name="ct_ps", bufs=1, space="PSUM"))

        # ---- resident constants ----
        wpre_sb = consts.tile([P, KC, QKV_DIM], dt.bfloat16)
        for k in range(KC):
            nc.sync.dma_start(wpre_sb[:, k, :], wpre_d[k * P:(k + 1) * P, :])
        wproj_sb = consts.tile([P, OUTPUT_DIM], dt.bfloat16)
        nc.sync.dma_start(wproj_sb[:], wproj_d[:, :])
        bpre_sb = consts.tile([P, QKV_DIM], dt.bfloat16)
        nc.sync.dma_start(bpre_sb[:], bpre_d[:, :])
        bproj_sb = consts.tile([P, OUTPUT_DIM], dt.float32)
        nc.sync.dma_start(bproj_sb[:], bproj_d[:, :])
        mask01_sb = consts.tile([P, P], dt.bfloat16)
        nc.sync.dma_start(mask01_sb[:], mask01_d[:, :])
        mask8_sb = consts.tile([P, 8], dt.bfloat16)
        nc.sync.dma_start(mask8_sb[:], mask8_d[:, :])
        ident_sb = consts.tile([P, P], dt.bfloat16)
        nc.sync.dma_start(ident_sb[:], ident_d[:, :])

        state = {}

        def emit_front(t):
            r0 = t * P
            # x^T tiles via XBAR DMA transpose: xt[d, kc, b] = x[r0+b, kc*128+d]
            xt = xt_pool.tile([P, KC, P], dt.bfloat16, name="xt")
            nc.sync.dma_start_transpose(xt[:], x_d[r0:r0 + P, :])

            qkv_sb = qkv_pool.tile([P, QKV_DIM], dt.bfloat16, name="qkv_sb")
            for s in range(NSWEEPS):
                chunks = []
                for c in range(SWEEP):
                    chunks.append(qkv_ps.tile([P, NCHUNK], dt.float32,
                                              name="qkvps", tag="qkvps"))
                for k in range(KC):
                    for c in range(SWEEP):
                        j0 = (s * SWEEP + c) * NCHUNK
                        nc.tensor.matmul(
                            chunks[c][:],
                            lhsT=xt[:, k, :],
                            rhs=wpre_sb[:, k, j0:j0 + NCHUNK],
                            start=(k == 0),
                            stop=(k == KC - 1),
                        )
                for c in range(SWEEP):
                    j0 = (s * SWEEP + c) * NCHUNK
                    # psum fp32 + b_pre -> bf16 SBUF
                    nc.vector.tensor_tensor(
                        qkv_sb[:, j0:j0 + NCHUNK],
                        chunks[c][:],
                        bpre_sb[:, j0:j0 + NCHUNK],
                        Alu.add,
                    )

            # transposed q/k in b-major layout: qt[d, b*16+h] = q[b, h*128+d],
            # so every 8-row group is a contiguous 128-column slice (matmul
            # operands must have a single free dimension). Copies alternate
            # ACT/DVE to balance engine load.
            qt = qt_pool.tile([P, P, HEAD_NUM], dt.bfloat16, name="qt")
            kt = kt_pool.tile([P, P, HEAD_NUM], dt.bfloat16, name="kt")
            for h in range(HEAD_NUM):
                tpq = z_ps.tile([P, P], dt.bfloat16, name="tp", tag="z8")
                nc.tensor.transpose(tpq[:], qkv_sb[:, h * P:(h + 1) * P],
                                    ident_sb[:])
                (nc.scalar.copy if h % 2 == 0 else nc.vector.tensor_copy)(
                    qt[:, :, h], tpq[:])
                tpk = z_ps.tile([P, P], dt.bfloat16, name="tp", tag="z8")
                nc.tensor.transpose(
                    tpk[:], qkv_sb[:, 2048 + h * P:2048 + (h + 1) * P],
                    ident_sb[:])
                (nc.scalar.copy if h % 2 == 1 else nc.vector.tensor_copy)(
                    kt[:, :, h], tpk[:])

            # v8[(b_loc, g), grp, d] = v[8*grp + b_loc, g*128 + d]
            v8 = v8_pool.tile([P, GROUPS, HEAD_DIM], dt.bfloat16, name="v8")
            for g in range(GROUPS):
                nc.sync.dma_start(
                    v8[:, g, :],
                    qkv_sb[8 * g:8 * g + 8, 4096:6144].rearrange(
                        "b (g d) -> b g d", d=HEAD_DIM),
                )
            state[t] = (qt, kt, v8)

        def emit_back(t):
            qt, kt, v8 = state.pop(t)
            r0 = t * P
            ct = ct_ps.tile([P, P], dt.float32, name="ct")
            for g in range(GROUPS):
                b0 = 8 * g
                z8 = z_ps.tile([P, P], dt.float32, name="z8", tag="z8")
                # scores for 8 rows x all head pairs: [(b,h), (b',g)]
                nc.tensor.matmul(
                    z8[:],
                    lhsT=qt[:, b0:b0 + 8, :].rearrange("d b h -> d (b h)"),
                    rhs=kt[:, b0:b0 + 8, :].rearrange("d b h -> d (b h)"),
                    start=True,
                    stop=True,
                )
                em_raw = att_pool.tile([P, P], dt.bfloat16, tag="emr",
                                       name="em_raw")
                nc.scalar.activation(em_raw[:], z8[:], Act.Exp,
                                     scale=INV_SQRT_D)
                em = att_pool.tile([P, P], dt.bfloat16, tag="em", name="em")
                den = att_pool.tile([P, 1], dt.float32, tag="den", name="den")
                # mask off cross-row blocks + row-sum (softmax denominator)
                nc.vector.scalar_tensor_tensor(
                    out=em[:], in0=em_raw[:], scalar=1.0, in1=mask01_sb[:],
                    op0=Alu.mult, op1=Alu.mult, accum_out=den[:])
                r32 = att_pool.tile([P, 1], dt.float32, tag="r32", name="r32")
                nc.vector.reciprocal(r32[:], den[:])
                rbf = att_pool.tile([P, 1], dt.bfloat16, tag="rbf", name="rbf")
                nc.scalar.copy(rbf[:], r32[:])
                # sigma[(b,g)] = sum_{(b,h)} em[(b,h),(b,g)] * r[(b,h)]
                sig = z8[:, 0:1]  # reuse the (now dead) z8 psum bank
                nc.tensor.matmul(sig, lhsT=em[:], rhs=rbf[:],
                                 start=True, stop=True)
                sd = att_pool.tile([P, 8], dt.bfloat16, tag="sd", name="sd")
                nc.vector.tensor_scalar(sd[:], mask8_sb[:], sig, None, Alu.mult)
                # C^T[:, rows of this group] += sigma-weighted V rows
                nc.tensor.matmul(ct[:, b0:b0 + 8], lhsT=v8[:, g, :], rhs=sd[:],
                                 start=True, stop=True)

            ct_sb = ct_pool.tile([P, P], dt.bfloat16, name="ct_sb")
            nc.scalar.copy(ct_sb[:], ct[:])

            out_sb = out_pool.tile([P, OUTPUT_DIM], dt.float32, name="out_sb")
            for c in range(OUTPUT_DIM // NCHUNK):
                o_ps = qkv_ps.tile([P, NCHUNK], dt.float32, name="o_ps",
                                   tag="qkvps")
                nc.tensor.matmul(
                    o_ps[:],
                    lhsT=ct_sb[:],
                    rhs=wproj_sb[:, c * NCHUNK:(c + 1) * NCHUNK],
                    start=True,
                    stop=True,
                )
                nc.vector.tensor_tensor(
                    out_sb[:, c * NCHUNK:(c + 1) * NCHUNK],
                    o_ps[:],
                    bproj_sb[:, c * NCHUNK:(c + 1) * NCHUNK],
                    Alu.add,
                )
            nc.sync.dma_start(out_d[r0:r0 + P, :], out_sb[:])

        # software pipeline: tile t's attention runs while tile t+1's qkv
        # matmuls keep the PE busy and the transpose copies drain on ACT/DVE.
        for t in range(n_tiles):
            emit_front(t)
            if t > 0:
                emit_back(t - 1)
        emit_back(n_tiles - 1)

    nc.compile()
    return nc


def _host_inputs(x, W_pre, b_pre, W_proj, b_proj, n_tiles=ROWS_PER_CORE // P,
                 n_cores=N_CORES):
    rows = n_tiles * P
    xf = np.ascontiguousarray(np.asarray(x, dtype=np.float32)
                              .reshape(-1, INPUT_DIM)).astype(BF16)
    wpre16 = np.asarray(W_pre, dtype=np.float32).astype(BF16)
    wproj16 = np.asarray(W_proj, dtype=np.float32).astype(BF16)
    bpre_rep = np.broadcast_to(
        np.asarray(b_pre, dtype=np.float32).astype(BF16)[None, :],
        (P, QKV_DIM)).copy()
    bproj_rep = np.broadcast_to(
        (16.0 * np.asarray(b_proj, dtype=np.float32))[None, :],
        (P, OUTPUT_DIM)).copy()
    pi = np.arange(P)[:, None] // HEAD_NUM
    fi = np.arange(P)[None, :] // HEAD_NUM
    mask01 = (pi == fi).astype(BF16)
    mask8 = (np.arange(P)[:, None] // HEAD_NUM
             == np.arange(8)[None, :]).astype(BF16)
    ident = np.eye(P).astype(BF16)

    in_maps = []
    for c in range(n_cores):
        in_maps.append({
            "x": np.ascontiguousarray(xf[c * rows:(c + 1) * rows]),
            "w_pre": wpre16,
            "b_pre_rep": bpre_rep,
            "w_proj": wproj16,
            "b_proj16_rep": bproj_rep,
            "mask01": mask01,
            "mask8": mask8,
            "ident": ident,
        })
    return in_maps


def kernel(x, W_pre, b_pre, W_proj, b_proj):
    global _PROG
    from concourse.bass_utils import run_bass_kernel_spmd

    if _PROG is None:
        _PROG = _build_program()

    in_maps = _host_inputs(x, W_pre, b_pre, W_proj, b_proj)
    res = run_bass_kernel_spmd(_PROG, in_maps, list(range(N_CORES)))
    out = np.concatenate([res.results[c]["out"] for c in range(N_CORES)],
                         axis=0)
    return out.reshape(*np.asarray(x).shape[:-1], OUTPUT_DIM).astype(np.float32)


# ---------------------------------------------------------------------------
# Dev/benchmark helpers (not used by the grading path).
# ---------------------------------------------------------------------------

def _make_sharded_fn(nc, n_cores=N_CORES):
    """Replicates bass2jax.run_bass_via_pjrt's multi-core path but without
    donation, returning (fn, in_names, out_info) so inputs can be staged on
    device once and execution timed across repeated calls."""
    import jax
    from jax.sharding import Mesh, PartitionSpec, NamedSharding
    from jax.experimental.shard_map import shard_map
    from concourse import mybir
    from concourse.bass2jax import (_bass_exec_p, install_neuronx_cc_hook,
                                    partition_id_tensor)

    install_neuronx_cc_hook()
    in_names, out_names, out_avals = [], [], []
    for alloc in nc.m.functions[0].allocations:
        if not isinstance(alloc, mybir.MemoryLocationSet):
            continue
        name = alloc.memorylocations[0].name
        if alloc.kind == "ExternalInput":
            in_names.append(name)
        elif alloc.kind == "ExternalOutput":
            out_names.append(name)
            out_avals.append(jax.core.ShapedArray(
                tuple(alloc.tensor_shape), mybir.dt.np(alloc.dtype)))
    partition_name = (nc.partition_id_tensor.name
                      if nc.partition_id_tensor else None)
    if partition_name in in_names:
        in_names.remove(partition_name)
    n_params = len(in_names)
    all_names = list(in_names) + list(out_names)
    if partition_name is not None:
        all_names.append(partition_name)

    def _body(*args):
        operands = list(args)
        if partition_name is not None:
            operands.append(partition_id_tensor())
        return tuple(_bass_exec_p.bind(
            *operands,
            out_avals=tuple(out_avals),
            in_names=tuple(all_names),
            out_names=tuple(out_names),
            lowering_input_output_aliases=(),
            sim_require_finite=True,
            sim_require_nnan=True,
            nc=nc,
        ))

    devices = jax.devices()[:n_cores]
    mesh = Mesh(np.asarray(devices), ("core",))
    spec = PartitionSpec("core")
    fn = jax.jit(shard_map(_body, mesh=mesh,
                           in_specs=(spec,) * (n_params + len(out_names)),
                           out_specs=(spec,) * len(out_names),
                           check_rep=False))
    sharding = NamedSharding(mesh, spec)
    return fn, in_names, out_names, out_avals, sharding


def run_timed(nc, in_maps, iters=10):
    """Stage inputs on device, run `iters` times, return (results, times)."""
    import time as _time
    import jax

    n_cores = len(in_maps)
    fn, in_names, out_names, out_avals, sharding = _make_sharded_fn(nc, n_cores)
    dev_in = [
        jax.device_put(
            np.concatenate([np.asarray(in_maps[c][nm])
                            for c in range(n_cores)], axis=0), sharding)
        for nm in in_names
    ]
    dev_zero = [
        jax.device_put(
            np.zeros((n_cores * av.shape[0], *av.shape[1:]), av.dtype),
            sharding)
        for av in out_avals
    ]
    outs = fn(*dev_in, *dev_zero)
    jax.block_until_ready(outs)
    times = []
    for _ in range(iters):
        t0 = _time.perf_counter()
        outs = fn(*dev_in, *dev_zero)
        jax.block_until_ready(outs)
        times.append(_time.perf_counter() - t0)
    results = [
        {nm: np.asarray(outs[i]).reshape(n_cores, *out_avals[i].shape)[c]
         for i, nm in enumerate(out_names)}
        for c in range(n_cores)
    ]
    return results, times


# revision 26
# speedup vs baseline: 84.6894x; 84.6894x over previous
"""Trainium2 Bass kernel for the nn_Attention problem.

Math (per flattened batch row b of x):
    qkv = x @ W_pre + b_pre                  # [B, 3*16*128]
    q,k,v -> [B, 16, 128]
    S = softmax(q k^T / sqrt(128), axis=g)   # [B, 16, 16]
    out = (sum_h S_h) . v @ W_proj + 16*b_proj
        = (sigma^T V) @ W_proj + 16*b_proj   with sigma[g] = sum_h S[h, g]

Implementation notes:
  - Data-parallel over 8 NeuronCores: 4096 rows/core (32 tiles of 128 rows).
  - bf16 matmuls with fp32 PSUM accumulation; softmax in fp32.
  - Attention processed in groups of 8 rows so the 128x128 PE array is full:
    stationary/moving operands are contiguous [d=128, (8 rows x 16 heads)]
    slices of PE-transposed, b-major Q/K. Cross-row score blocks are zeroed
    by a multiplicative block-diagonal mask fused with the softmax-denominator
    row sum on the DVE.
  - sigma = E^T r computed on the PE (contract over the (row,head) partition
    dim), scattered to a block-diagonal [128, 8] operand, and contracted with
    V8 (v rows expanded onto partitions via an SBUF->SBUF reshape DMA) to
    accumulate C^T directly; C^T is then the stationary operand of the final
    projection matmul.
"""

import sys

import numpy as np

for _p in ("/opt/trn_rl_repo",):
    if _p not in sys.path:
        sys.path.insert(0, _p)

import ml_dtypes  # noqa: E402

BF16 = ml_dtypes.bfloat16

HEAD_NUM = 16
HEAD_DIM = 128
INPUT_DIM = 1024
OUTPUT_DIM = 1024
QKV_DIM = 3 * HEAD_NUM * HEAD_DIM  # 6144
N_CORES = 8
B_TOTAL = 64 * 512
ROWS_PER_CORE = B_TOTAL // N_CORES  # 4096
P = 128

_PROG = None


def _build_program(n_tiles=ROWS_PER_CORE // P, repeats=1):
    from contextlib import ExitStack

    import concourse.tile as tile
    from concourse import bacc, mybir

    dt = mybir.dt
    Alu = mybir.AluOpType
    Act = mybir.ActivationFunctionType

    rows = n_tiles * P
    nc = bacc.Bacc("TRN2", target_bir_lowering=False, debug=False,
                   num_devices=N_CORES)

    x_d = nc.dram_tensor("x", [rows, INPUT_DIM], dt.bfloat16,
                         kind="ExternalInput")
    wpre_d = nc.dram_tensor("w_pre", [INPUT_DIM, QKV_DIM], dt.bfloat16,
                            kind="ExternalInput")
    bpre_d = nc.dram_tensor("b_pre_rep", [P, QKV_DIM], dt.bfloat16,
                            kind="ExternalInput")
    wproj_d = nc.dram_tensor("w_proj", [HEAD_DIM, OUTPUT_DIM], dt.bfloat16,
                             kind="ExternalInput")
    bproj_d = nc.dram_tensor("b_proj16_rep", [P, OUTPUT_DIM], dt.float32,
                             kind="ExternalInput")
    mask01_d = nc.dram_tensor("mask01", [P, P], dt.bfloat16,
                              kind="ExternalInput")
    mask8_d = nc.dram_tensor("mask8", [P, 8], dt.bfloat16,
                             kind="ExternalInput")
    ident_d = nc.dram_tensor("ident", [P, P], dt.bfloat16,
                             kind="ExternalInput")
    out_d = nc.dram_tensor("out", [rows, OUTPUT_DIM], dt.float32,
                           kind="ExternalOutput")

    KC = INPUT_DIM // P          # 8 contraction chunks
    NCHUNK = 512                 # psum free width
    SWEEP = 3                    # psum banks used by the qkv matmul
    NSWEEPS = QKV_DIM // (SWEEP * NCHUNK)  # 4
    GROUPS = P // 8              # 16 groups of 8 rows per tile
    INV_SQRT_D = 1.0 / float(np.sqrt(HEAD_DIM))

    with tile.TileContext(nc) as tc, ExitStack() as ctx:
        consts = ctx.enter_context(tc.tile_pool(name="consts", bufs=1))
        xt_pool = ctx.enter_context(tc.tile_pool(name="xt", bufs=2))
        qkv_pool = ctx.enter_context(tc.tile_pool(name="qkv", bufs=2))
        qt_pool = ctx.enter_context(tc.tile_pool(name="qt", bufs=2))
        kt_pool = ctx.enter_context(tc.tile_pool(name="kt", bufs=2))
        v8_pool = ctx.enter_context(tc.tile_pool(name="v8", bufs=2))
        att_pool = ctx.enter_context(tc.tile_pool(name="att", bufs=4))
        ct_pool = ctx.enter_context(tc.tile_pool(name="ct", bufs=2))
        out_pool = ctx.enter_context(tc.tile_pool(name="outp", bufs=2))

        qkv_ps = ctx.enter_context(
            tc.tile_pool(name="qkv_ps", bufs=SWEEP, space="PSUM"))
        z_ps = ctx.enter_context(tc.tile_pool(name="z_ps", bufs=2, space="PSUM"))
        tp_ps = ctx.enter_context(tc.tile_pool(name="tp_ps", bufs=2, space="PSUM"))
        ct_ps = ctx.enter_context(tc.tile_pool(name="ct_ps", bufs=1, space="PSUM"))

        # tile 0's x^T load goes first so the first qkv matmul only waits
        # on it plus the first W chunk, not the whole 12.6MB weight load.
        xt0 = xt_pool.tile([P, KC, P], dt.bfloat16, name="xt")
        nc.sync.dma_start_transpose(xt0[:], x_d[0:P, :])
        preloaded_xt = {0: xt0}

        # ---- resident constants ----
        wpre_sb = consts.tile([P, KC, QKV_DIM], dt.bfloat16)
        for k in range(KC):
            eng = nc.sync if k % 2 == 0 else nc.scalar
            eng.dma_start(wpre_sb[:, k, :], wpre_d[k * P:(k + 1) * P, :])
        wproj_sb = consts.tile([P, OUTPUT_DIM], dt.bfloat16)
        nc.sync.dma_start(wproj_sb[:], wproj_d[:, :])
        bpre_sb = consts.tile([P, QKV_DIM], dt.bfloat16)
        nc.sync.dma_start(bpre_sb[:], bpre_d[:, :])
        bproj_sb = consts.tile([P, OUTPUT_DIM], dt.float32)
        nc.sync.dma_start(bproj_sb[:], bproj_d[:, :])
        mask01_sb = consts.tile([P, P], dt.bfloat16)
        nc.sync.dma_start(mask01_sb[:], mask01_d[:, :])
        mask8_sb = consts.tile([P, 8], dt.bfloat16)
        nc.sync.dma_start(mask8_sb[:], mask8_d[:, :])
        ident_sb = consts.tile([P, P], dt.bfloat16)
        nc.sync.dma_start(ident_sb[:], ident_d[:, :])

        state = {}

        def front_gen(t):
            """Emits tile t's qkv matmuls, yielding after each one so the
            caller can interleave the previous tile's attention steps into
            the PE queue; finishes with transposes + the V8 reshape."""
            r0 = t * P
            # x^T tiles via XBAR DMA transpose: xt[d, kc, b] = x[r0+b, kc*128+d]
            if t in preloaded_xt:
                xt = preloaded_xt.pop(t)
            else:
                xt = xt_pool.tile([P, KC, P], dt.bfloat16, name="xt")
                nc.sync.dma_start_transpose(xt[:], x_d[r0:r0 + P, :])

            qkv_sb = qkv_pool.tile([P, QKV_DIM], dt.bfloat16, name="qkv_sb")
            for s in range(NSWEEPS):
                chunks = []
                for c in range(SWEEP):
                    chunks.append(qkv_ps.tile([P, NCHUNK], dt.float32,
                                              name="qkvps", tag="qkvps"))
                for k in range(KC):
                    for c in range(SWEEP):
                        j0 = (s * SWEEP + c) * NCHUNK
                        nc.tensor.matmul(
                            chunks[c][:],
                            lhsT=xt[:, k, :],
                            rhs=wpre_sb[:, k, j0:j0 + NCHUNK],
                            start=(k == 0),
                            stop=(k == KC - 1),
                        )
                        yield
                for c in range(SWEEP):
                    j0 = (s * SWEEP + c) * NCHUNK
                    # psum fp32 + b_pre -> bf16 SBUF
                    nc.vector.tensor_tensor(
                        qkv_sb[:, j0:j0 + NCHUNK],
                        chunks[c][:],
                        bpre_sb[:, j0:j0 + NCHUNK],
                        Alu.add,
                    )

            # transposed q/k in b-major layout: qt[d, b*16+h] = q[b, h*128+d],
            # so every 8-row group is a contiguous 128-column slice (matmul
            # operands must have a single free dimension). 8 head-transposes
            # share one PSUM bank, drained by a single strided copy.
            qt = qt_pool.tile([P, P, HEAD_NUM], dt.bfloat16, name="qt")
            kt = kt_pool.tile([P, P, HEAD_NUM], dt.bfloat16, name="kt")
            for qk in range(2):
                src_off = 2048 * qk
                dst = (qt, kt)[qk]
                for hb in range(2):
                    h0 = 8 * hb
                    tpb = tp_ps.tile([P, 8 * P], dt.bfloat16, name="tpb",
                                     tag="tpb")
                    for hl in range(8):
                        nc.tensor.transpose(
                            tpb[:, hl * P:(hl + 1) * P],
                            qkv_sb[:, src_off + (h0 + hl) * P:
                                   src_off + (h0 + hl + 1) * P],
                            ident_sb[:])
                    (nc.scalar.copy if (qk + hb) % 2 == 0
                     else nc.vector.tensor_copy)(
                        dst[:, :, h0:h0 + 8],
                        tpb.rearrange("d (h b) -> d b h", b=P))

            # v8[(b_loc, g), grp, d] = v[8*grp + b_loc, g*128 + d]
            v8 = v8_pool.tile([P, GROUPS, HEAD_DIM], dt.bfloat16, name="v8")
            for g in range(GROUPS):
                nc.sync.dma_start(
                    v8[:, g, :],
                    qkv_sb[8 * g:8 * g + 8, 4096:6144].rearrange(
                        "b (g d) -> b g d", d=HEAD_DIM),
                )
            state[t] = (qt, kt, v8)

        def attention_steps(t):
            """Returns (steps, tail): `steps` are callables interleaved with
            the next tile's qkv matmuls; MM2 lags MM1 by one group and MM3 by
            two so the PE never waits inside one group's softmax chain."""
            qt, kt, v8 = state.pop(t)
            r0 = t * P
            ct_box = {}
            zs, ems, rbfs, sds = {}, {}, {}, {}

            def mm1(g):
                if g == 0:
                    ct_box["ct"] = ct_ps.tile([P, P], dt.float32, name="ct")
                b0 = 8 * g
                z8 = z_ps.tile([P, P], dt.float32, name="z8", tag="z8")
                # scores for 8 rows x all head pairs: [(b,h), (b',g)]
                nc.tensor.matmul(
                    z8[:],
                    lhsT=qt[:, b0:b0 + 8, :].rearrange("d b h -> d (b h)"),
                    rhs=kt[:, b0:b0 + 8, :].rearrange("d b h -> d (b h)"),
                    start=True,
                    stop=True,
                )
                zs[g] = z8
                em_raw = att_pool.tile([P, P], dt.bfloat16, tag="emr",
                                       name="em_raw")
                nc.scalar.activation(em_raw[:], z8[:], Act.Exp,
                                     scale=INV_SQRT_D)
                em = att_pool.tile([P, P], dt.bfloat16, tag="em", name="em")
                den = att_pool.tile([P, 1], dt.float32, tag="den", name="den")
                # mask off cross-row blocks + row-sum (softmax denominator)
                nc.vector.scalar_tensor_tensor(
                    out=em[:], in0=em_raw[:], scalar=1.0, in1=mask01_sb[:],
                    op0=Alu.mult, op1=Alu.mult, accum_out=den[:])
                ems[g] = em
                r32 = att_pool.tile([P, 1], dt.float32, tag="r32", name="r32")
                nc.vector.reciprocal(r32[:], den[:])
                rbf = att_pool.tile([P, 1], dt.bfloat16, tag="rbf", name="rbf")
                nc.vector.tensor_copy(rbf[:], r32[:])
                rbfs[g] = rbf

            def mm2(g):
                # sigma[(b,g)] = sum_{(b,h)} em[(b,h),(b,g)] * r[(b,h)]
                sig = zs[g][:, 0:1]  # reuse the (now dead) z8 psum bank
                nc.tensor.matmul(sig, lhsT=ems.pop(g)[:], rhs=rbfs.pop(g)[:],
                                 start=True, stop=True)
                sd = att_pool.tile([P, 8], dt.bfloat16, tag="sd", name="sd")
                nc.vector.tensor_scalar(sd[:], mask8_sb[:], sig, None, Alu.mult)
                sds[g] = sd
                zs.pop(g)

            def mm3(g):
                b0 = 8 * g
                # C^T[:, rows of this group] += sigma-weighted V rows
                nc.tensor.matmul(ct_box["ct"][:, b0:b0 + 8], lhsT=v8[:, g, :],
                                 rhs=sds.pop(g)[:], start=True, stop=True)

            steps = []
            for g in range(GROUPS + 3):
                def step(g=g):
                    # mm2 before mm1 so the z8 slot freed by mm2(g-2) is
                    # available for mm1(g)'s allocation in the same step.
                    if 2 <= g <= GROUPS + 1:
                        mm2(g - 2)
                    if g < GROUPS:
                        mm1(g)
                    if g >= 3:
                        mm3(g - 3)
                steps.append(step)

            def tail():
                ct_sb = ct_pool.tile([P, P], dt.bfloat16, name="ct_sb")
                nc.scalar.copy(ct_sb[:], ct_box["ct"][:])
                out_sb = out_pool.tile([P, OUTPUT_DIM], dt.float32,
                                       name="out_sb")
                for c in range(OUTPUT_DIM // NCHUNK):
                    o_ps = qkv_ps.tile([P, NCHUNK], dt.float32, name="o_ps",
                                       tag="qkvps")
                    nc.tensor.matmul(
                        o_ps[:],
                        lhsT=ct_sb[:],
                        rhs=wproj_sb[:, c * NCHUNK:(c + 1) * NCHUNK],
                        start=True,
                        stop=True,
                    )
                    nc.vector.tensor_tensor(
                        out_sb[:, c * NCHUNK:(c + 1) * NCHUNK],
                        o_ps[:],
                        bproj_sb[:, c * NCHUNK:(c + 1) * NCHUNK],
                        Alu.add,
                    )
                nc.sync.dma_start(out_d[r0:r0 + P, :], out_sb[:])

            return steps, tail

        # software pipeline: tile t's attention steps are interleaved into
        # tile t+1's qkv matmul stream so the PE stays busy while the
        # softmax chains drain on ACT/DVE. repeats>1 re-runs the whole pass
        # (same outputs) for benchmarking.
        prev = None
        for _r in range(repeats):
            for t in range(n_tiles):
                steps, tail = attention_steps(prev) if prev is not None \
                    else ([], None)
                si = 0
                yi = 0
                for _ in front_gen(t):
                    yi += 1
                    if si < len(steps) and yi % 5 == 0:
                        steps[si]()
                        si += 1
                while si < len(steps):
                    steps[si]()
                    si += 1
                if tail is not None:
                    tail()
                prev = t
        steps, tail = attention_steps(prev)
        for s in steps:
            s()
        tail()

    nc.compile()
    return nc


def _host_inputs(x, W_pre, b_pre, W_proj, b_proj, n_tiles=ROWS_PER_CORE // P,
                 n_cores=N_CORES):
    rows = n_tiles * P
    xf = np.ascontiguousarray(np.asarray(x, dtype=np.float32)
                              .reshape(-1, INPUT_DIM)).astype(BF16)
    wpre16 = np.asarray(W_pre, dtype=np.float32).astype(BF16)
    wproj16 = np.asarray(W_proj, dtype=np.float32).astype(BF16)
    bpre_rep = np.broadcast_to(
        np.asarray(b_pre, dtype=np.float32).astype(BF16)[None, :],
        (P, QKV_DIM)).copy()
    bproj_rep = np.broadcast_to(
        (16.0 * np.asarray(b_proj, dtype=np.float32))[None, :],
        (P, OUTPUT_DIM)).copy()
    pi = np.arange(P)[:, None] // HEAD_NUM
    fi = np.arange(P)[None, :] // HEAD_NUM
    mask01 = (pi == fi).astype(BF16)
    mask8 = (np.arange(P)[:, None] // HEAD_NUM
             == np.arange(8)[None, :]).astype(BF16)
    ident = np.eye(P).astype(BF16)

    in_maps = []
    for c in range(n_cores):
        in_maps.append({
            "x": np.ascontiguousarray(xf[c * rows:(c + 1) * rows]),
            "w_pre": wpre16,
            "b_pre_rep": bpre_rep,
            "w_proj": wproj16,
            "b_proj16_rep": bproj_rep,
            "mask01": mask01,
            "mask8": mask8,
            "ident": ident,
        })
    return in_maps


def kernel(x, W_pre, b_pre, W_proj, b_proj):
    global _PROG
    from concourse.bass_utils import run_bass_kernel_spmd

    if _PROG is None:
        _PROG = _build_program()

    in_maps = _host_inputs(x, W_pre, b_pre, W_proj, b_proj)
    res = run_bass_kernel_spmd(_PROG, in_maps, list(range(N_CORES)))
    out = np.concatenate([res.results[c]["out"] for c in range(N_CORES)],
                         axis=0)
    return out.reshape(*np.asarray(x).shape[:-1], OUTPUT_DIM).astype(np.float32)


# ---------------------------------------------------------------------------
# Dev/benchmark helpers (not used by the grading path).
# ---------------------------------------------------------------------------

def _make_sharded_fn(nc, n_cores=N_CORES):
    """Replicates bass2jax.run_bass_via_pjrt's multi-core path but without
    donation, returning (fn, in_names, out_info) so inputs can be staged on
    device once and execution timed across repeated calls."""
    import jax
    from jax.sharding import Mesh, PartitionSpec, NamedSharding
    from jax.experimental.shard_map import shard_map
    from concourse import mybir
    from concourse.bass2jax import (_bass_exec_p, install_neuronx_cc_hook,
                                    partition_id_tensor)

    install_neuronx_cc_hook()
    in_names, out_names, out_avals = [], [], []
    for alloc in nc.m.functions[0].allocations:
        if not isinstance(alloc, mybir.MemoryLocationSet):
            continue
        name = alloc.memorylocations[0].name
        if alloc.kind == "ExternalInput":
            in_names.append(name)
        elif alloc.kind == "ExternalOutput":
            out_names.append(name)
            out_avals.append(jax.core.ShapedArray(
                tuple(alloc.tensor_shape), mybir.dt.np(alloc.dtype)))
    partition_name = (nc.partition_id_tensor.name
                      if nc.partition_id_tensor else None)
    if partition_name in in_names:
        in_names.remove(partition_name)
    n_params = len(in_names)
    all_names = list(in_names) + list(out_names)
    if partition_name is not None:
        all_names.append(partition_name)

    def _body(*args):
        operands = list(args)
        if partition_name is not None:
            operands.append(partition_id_tensor())
        return tuple(_bass_exec_p.bind(
            *operands,
            out_avals=tuple(out_avals),
            in_names=tuple(all_names),
            out_names=tuple(out_names),
            lowering_input_output_aliases=(),
            sim_require_finite=True,
            sim_require_nnan=True,
            nc=nc,
        ))

    devices = jax.devices()[:n_cores]
    mesh = Mesh(np.asarray(devices), ("core",))
    spec = PartitionSpec("core")
    fn = jax.jit(shard_map(_body, mesh=mesh,
                           in_specs=(spec,) * (n_params + len(out_names)),
                           out_specs=(spec,) * len(out_names),
                           check_rep=False))
    sharding = NamedSharding(mesh, spec)
    return fn, in_names, out_names, out_avals, sharding


def run_timed(nc, in_maps, iters=10):
    """Stage inputs on device, run `iters` times, return (results, times)."""
    import time as _time
    import jax

    n_cores = len(in_maps)
    fn, in_names, out_names, out_avals, sharding = _make_sharded_fn(nc, n_cores)
    dev_in = [
        jax.device_put(
            np.concatenate([np.asarray(in_maps[c][nm])
                            for c in range(n_cores)], axis=0), sharding)
        for nm in in_names
    ]
    dev_zero = [
        jax.device_put(
            np.zeros((n_cores * av.shape[0], *av.shape[1:]), av.dtype),
            sharding)
        for av in out_avals
    ]
    outs = fn(*dev_in, *dev_zero)
    jax.block_until_ready(outs)
    times = []
    for _ in range(iters):
        t0 = _time.perf_counter()
        outs = fn(*dev_in, *dev_zero)
        jax.block_until_ready(outs)
        times.append(_time.perf_counter() - t0)
    results = [
        {nm: np.asarray(outs[i]).reshape(n_cores, *out_avals[i].shape)[c]
         for i, nm in enumerate(out_names)}
        for c in range(n_cores)
    ]
    return results, times
